# revision 4
# baseline (speedup 1.0000x reference)
"""GQA causal attention block (B=4, S=1024, D=4096, H=32, KH=8, HD=128) on 8
Trainium2 NeuronCores.

Sharding: tensor-parallel over heads (2-way) x data-parallel over batch
(4-way). Core c handles batch c//2 with q-heads [16*(c%2), 16*(c%2)+16) and
kv-heads [4*(c%2), 4*(c%2)+4) over the full 1024-token sequence. Each core
produces a partial output (its heads' contribution through its wo row-slice);
the host sums the two partials per batch. No device collectives, no K/V
duplication, no fully-masked attention tiles: queries in token-group g only
attend key tiles 0..4g+3 (causal skipping, uniform across cores).

Everything stays in SBUF: x arrives pre-transposed from the host ([D, S]
fp16), Q/K/V and attention outputs are SBUF-resident (no DRAM spills, no
on-device transposes). V is projected directly into [token, head-dim] layout
by making x the stationary matmul operand. cos/sin RoPE tables are computed
on host.

RoPE trick (as baseline): wq/wk columns host-permuted per head to
[even|odd] halves; rot(q) = q*cos + (S^T q)*sin with a constant 128x128
swap matrix via one matmul per head-chunk.

Matmuls run in fp16; softmax statistics in fp32. Scores biased by -8 before
exp (folded into mask / bias vector); the softmax division cancels it.
"""

import numpy as np

import concourse.bass as bass
import concourse.tile as tile
from concourse import bacc, mybir
from concourse.bass_utils import run_bass_kernel_spmd

B, S, D = 4, 1024, 4096
H, KH, HD = 32, 8, 128
N_CORES = 8
TP = 2                          # head-parallel ways
HPC = H // TP                   # 16 q heads per core
KHPC = KH // TP                 # 4 kv heads per core
TG = 512                        # token-group size
G = S // TG                     # 2 token groups
SCALE = 1.0 / float(np.sqrt(HD))
EXP_BIAS = -8.0
NEG = -1e9

MM_DT = mybir.dt.float16
MM_NP = np.float16
F32 = mybir.dt.float32
BF16 = mybir.dt.bfloat16

DT = D // 128                   # 32 d-tiles
DB = 4                          # weight DMA chunks per 512-col group
WCH = DT // DB                  # 8 d-tiles per weight chunk

_compiled = None


def _build():
    nc = bacc.Bacc("TRN2", target_bir_lowering=False, debug=False,
                   num_devices=N_CORES)

    xT = nc.dram_tensor("xT", [D, S], MM_DT, kind="ExternalInput").ap()
    wq = nc.dram_tensor("wq", [D, HPC * HD], MM_DT, kind="ExternalInput").ap()
    wk = nc.dram_tensor("wk", [D, KHPC * HD], MM_DT, kind="ExternalInput").ap()
    wv = nc.dram_tensor("wv", [D, KHPC * HD], MM_DT, kind="ExternalInput").ap()
    wo = nc.dram_tensor("wo", [HPC * HD, D], MM_DT, kind="ExternalInput").ap()
    maskT_d = nc.dram_tensor("maskT", [TG, TG], BF16, kind="ExternalInput").ap()
    cosT_d = nc.dram_tensor("cosT", [128, S], F32, kind="ExternalInput").ap()
    sinT_d = nc.dram_tensor("sinT", [128, S], F32, kind="ExternalInput").ap()
    rotT_d = nc.dram_tensor("rotT", [128, 128], MM_DT, kind="ExternalInput").ap()
    out = nc.dram_tensor("out", [S, D], F32, kind="ExternalOutput").ap()

    from contextlib import ExitStack

    es = ExitStack()
    with tile.TileContext(nc) as tc, es:
        const = es.enter_context(tc.tile_pool(name="const", bufs=1))
        trig = es.enter_context(tc.tile_pool(name="trig", bufs=1))
        maskp = es.enter_context(tc.tile_pool(name="maskp", bufs=1))
        xp = es.enter_context(tc.tile_pool(name="xp", bufs=1))
        qp = es.enter_context(tc.tile_pool(name="qp", bufs=2))
        kp = es.enter_context(tc.tile_pool(name="kp", bufs=1))
        vp = es.enter_context(tc.tile_pool(name="vp", bufs=1))
        atp = es.enter_context(tc.tile_pool(name="atp", bufs=2))
        wbuf = es.enter_context(tc.tile_pool(name="wbuf", bufs=2))
        pw = es.enter_context(tc.tile_pool(name="pw", bufs=3))
        sw = es.enter_context(tc.tile_pool(name="sw", bufs=2))
        pr = es.enter_context(tc.tile_pool(name="pr", bufs=3))
        rb = es.enter_context(tc.tile_pool(name="rb", bufs=1))
        ow = es.enter_context(tc.tile_pool(name="ow", bufs=2))
        ps_acc = es.enter_context(tc.tile_pool(name="ps_acc", bufs=5, space="PSUM"))
        ps_sc = es.enter_context(tc.tile_pool(name="ps_sc", bufs=3, space="PSUM"))

        # ---- constants / tables ----
        rotT = const.tile([128, 128], MM_DT, tag="rot")
        nc.sync.dma_start(out=rotT, in_=rotT_d)
        ones = const.tile([128, 128], MM_DT, tag="ones")
        nc.vector.memset(ones, 1.0)
        m8_t = const.tile([128, 1], F32, tag="m8")
        nc.vector.memset(m8_t, EXP_BIAS)

        mask_t = maskp.tile([128, TG // 128, TG], BF16, tag="mask")
        nc.sync.dma_start(
            out=mask_t, in_=maskT_d.rearrange("(t p) q -> p t q", p=128))

        cosT = trig.tile([128, S], F32, tag="cos")
        nc.sync.dma_start(out=cosT, in_=cosT_d)
        sinT = trig.tile([128, S], F32, tag="sin")
        nc.sync.dma_start(out=sinT, in_=sinT_d)

        # ---- x (pre-transposed on host): [128, 32, 1024] ----
        x_t = xp.tile([128, DT, S], MM_DT, tag="x")
        for xc in range(4):
            nc.sync.dma_start(
                out=x_t[:, xc * 8:(xc + 1) * 8, :],
                in_=xT[xc * 1024:(xc + 1) * 1024, :].rearrange(
                    "(dt p) t -> p dt t", p=128))

        # ---- helpers ----
        def rope_evict(acc, cos_cols, sin_cols, dst):
            q_s = pw.tile([128, TG], MM_DT, tag="qs")
            nc.scalar.copy(q_s, acc)
            ps2 = ps_sc.tile([128, TG], F32, tag="sc")
            nc.tensor.matmul(ps2, rotT, q_s, start=True, stop=True)
            t1 = pw.tile([128, TG], F32, tag="t1")
            nc.vector.tensor_mul(t1, q_s, cos_cols)
            t2 = pw.tile([128, TG], F32, tag="t2")
            nc.vector.tensor_mul(t2, ps2, sin_cols)
            nc.vector.tensor_add(dst, t1, t2)

        def proj_cols(w_ap, jg, tok, accs, n_jj=4):
            """accs[jj] += w[:, jg*512+jj*128 : +128]^T @ x[:, tok] over all D."""
            for db in range(DB):
                w_t = wbuf.tile([128, WCH, 512], MM_DT, tag="w")
                nc.sync.dma_start(
                    out=w_t,
                    in_=w_ap[db * 1024:(db + 1) * 1024,
                             jg * 512:(jg + 1) * 512].rearrange(
                                 "(dt p) c -> p dt c", p=128))
                for dd in range(WCH):
                    d = db * WCH + dd
                    for jj in range(n_jj):
                        nc.tensor.matmul(
                            accs[jj], w_t[:, dd, jj * 128:(jj + 1) * 128],
                            x_t[:, d, tok], start=(d == 0), stop=(d == DT - 1))

        # ---- K projection + RoPE -> kT [128, 4, 1024] ----
        kT = kp.tile([128, KHPC, S], MM_DT, tag="k")
        for g in range(G):
            tok = slice(g * TG, (g + 1) * TG)
            accs = [ps_acc.tile([128, TG], F32, tag="acc", name=f"kacc{i}")
                    for i in range(KHPC)]
            proj_cols(wk, 0, tok, accs)
            for kh in range(KHPC):
                rope_evict(accs[kh], cosT[:, tok], sinT[:, tok],
                           kT[:, kh, tok])

        # ---- V projection (transposed: x stationary) -> v_n [128, 8, 512] ----
        # v_n[:, tt, :] = V rows for token-tile tt, cols = 4 kv heads * 128
        v_n = vp.tile([128, S // 128, KHPC * HD], MM_DT, tag="v")
        for ttg in range(2):
            accs = [ps_acc.tile([128, 512], F32, tag="acc", name=f"vacc{i}")
                    for i in range(4)]
            for db in range(DB):
                w_t = wbuf.tile([128, WCH, 512], MM_DT, tag="w")
                nc.sync.dma_start(
                    out=w_t,
                    in_=wv[db * 1024:(db + 1) * 1024, :].rearrange(
                        "(dt p) c -> p dt c", p=128))
                for dd in range(WCH):
                    d = db * WCH + dd
                    for tt in range(4):
                        tok = (ttg * 4 + tt) * 128
                        nc.tensor.matmul(
                            accs[tt], x_t[:, d, tok:tok + 128], w_t[:, dd, :],
                            start=(d == 0), stop=(d == DT - 1))
            for tt in range(4):
                nc.scalar.copy(v_n[:, ttg * 4 + tt, :], accs[tt])

        # ---- Q projection for one token group -> qT [128, 16, 512] ----
        def q_proj_group(g, qT, jgs):
            tok = slice(g * TG, (g + 1) * TG)
            for jg in jgs:
                accs = [ps_acc.tile([128, TG], F32, tag="acc", name=f"qacc{i}")
                        for i in range(4)]
                proj_cols(wq, jg, tok, accs)
                for jj in range(4):
                    rope_evict(accs[jj], cosT[:, tok], sinT[:, tok],
                               qT[:, jg * 4 + jj, :])

        # ---- attention for one (head, token-group) ----
        def attend_head(h, g, qT, attnT):
            kh = h // (HPC // KHPC)
            q_s = qT[:, h, :]
            ntau = 4 * (g + 1)
            oT_ps = ps_acc.tile([128, TG], F32, tag="acc", name="oT")
            sum_ps = ps_acc.tile([128, TG], F32, tag="acc", name="sum")
            for tau in range(ntau):
                sc_ps = ps_sc.tile([128, TG], F32, tag="sc")
                nc.tensor.matmul(
                    sc_ps, kT[:, kh, tau * 128:(tau + 1) * 128], q_s,
                    start=True, stop=True)
                p_t = pr.tile([128, TG], MM_DT, tag="pr")
                dt_ = tau - 4 * g
                if dt_ < 0:
                    # past 512-block: fully unmasked, bias -8
                    nc.scalar.activation(p_t, sc_ps,
                                         mybir.ActivationFunctionType.Exp,
                                         bias=m8_t, scale=SCALE)
                else:
                    # diagonal 512-block: causal mask tile (includes -8)
                    sc_s = sw.tile([128, TG], F32, tag="sw")
                    nc.vector.scalar_tensor_tensor(
                        out=sc_s, in0=sc_ps, scalar=SCALE,
                        in1=mask_t[:, dt_, :],
                        op0=mybir.AluOpType.mult, op1=mybir.AluOpType.add)
                    nc.scalar.activation(p_t, sc_s,
                                         mybir.ActivationFunctionType.Exp)
                nc.tensor.matmul(oT_ps, v_n[:, tau, kh * 128:(kh + 1) * 128],
                                 p_t, start=(tau == 0), stop=(tau == ntau - 1))
                nc.tensor.matmul(sum_ps, ones, p_t, start=(tau == 0),
                                 stop=(tau == ntau - 1))
            rB2 = rb.tile([128, TG], F32, tag="rb2")
            nc.vector.reciprocal_approx_fast(rB2, sum_ps)
            nc.vector.tensor_mul(attnT[:, h, :], oT_ps, rB2)

        # ---- output projection for one token group, one 512-col group ----
        def out_group(g, djg, attnT):
            accs = [ps_acc.tile([128, 512], F32, tag="acc", name=f"oacc{i}")
                    for i in range(4)]
            for hb in range(2):
                w_t = wbuf.tile([128, WCH, 512], MM_DT, tag="w")
                nc.sync.dma_start(
                    out=w_t,
                    in_=wo[hb * 1024:(hb + 1) * 1024,
                           djg * 512:(djg + 1) * 512].rearrange(
                               "(hh p) c -> p hh c", p=128))
                for hh in range(WCH):
                    hd = hb * WCH + hh
                    for t4 in range(4):
                        nc.tensor.matmul(
                            accs[t4], attnT[:, hd, t4 * 128:(t4 + 1) * 128],
                            w_t[:, hh, :], start=(hd == 0),
                            stop=(hd == HPC - 1))
            for t4 in range(4):
                o_s = ow.tile([128, 512], F32, tag="ow")
                nc.any.tensor_copy(o_s, accs[t4])
                nc.sync.dma_start(
                    out=out[g * TG + t4 * 128:g * TG + (t4 + 1) * 128,
                            djg * 512:(djg + 1) * 512],
                    in_=o_s)

        # ---- phase schedule ----
        qT0 = qp.tile([128, HPC, TG], MM_DT, tag="q", name="qT0")
        qT1 = qp.tile([128, HPC, TG], MM_DT, tag="q", name="qT1")
        attnT0 = atp.tile([128, HPC, TG], MM_DT, tag="at", name="at0")
        attnT1 = atp.tile([128, HPC, TG], MM_DT, tag="at", name="at1")

        # Q for g0
        q_proj_group(0, qT0, range(4))
        # attention g0 interleaved with Q projection g1
        for i in range(4):
            q_proj_group(1, qT1, [i])
            for h in range(4 * i, 4 * i + 4):
                attend_head(h, 0, qT0, attnT0)
        # attention g1 interleaved with wo for g0
        for i in range(8):
            out_group(0, i, attnT0)
            for h in range(2 * i, 2 * i + 2):
                attend_head(h, 1, qT1, attnT1)
        # wo for g1
        for djg in range(8):
            out_group(1, djg, attnT1)

    nc.compile()
    return nc


def _get_compiled():
    global _compiled
    if _compiled is None:
        _compiled = _build()
    return _compiled


def _ab_perm(n_heads):
    """Per-head column permutation to [even dims | odd dims]."""
    p = []
    for h in range(n_heads):
        base = h * HD
        p.extend(range(base, base + HD, 2))
        p.extend(range(base + 1, base + HD, 2))
    return np.asarray(p)


def _host_prep(x, freqs_cis, mask, wq, wk, wv, wo):
    """Shard per core: core c -> batch c//2, head-slice c%2."""
    import ml_dtypes

    x = np.asarray(x, dtype=np.float32)

    # RoPE tables [128, S]: rows 0:64 = trig(pos * inv_freq_i), rows 64:128 dup
    fr = np.asarray(freqs_cis, dtype=np.float64)      # [S, 64]
    cosT = np.empty((128, S), dtype=np.float32)
    sinT = np.empty((128, S), dtype=np.float32)
    cosT[0:64] = np.cos(fr).T
    cosT[64:128] = cosT[0:64]
    sinT[0:64] = np.sin(fr).T
    sinT[64:128] = sinT[0:64]

    # causal 512-block mask [keys, queries] with -8 exp-bias folded in
    kk, qq = np.meshgrid(np.arange(TG), np.arange(TG), indexing="ij")
    maskT = np.where(kk <= qq, np.float32(EXP_BIAS), np.float32(NEG))
    maskT = maskT.astype(ml_dtypes.bfloat16)

    # S^T for rot(q) = q*cos + (S^T q)*sin in [a|b] layout
    rotT = np.zeros((128, 128), dtype=MM_NP)
    rotT[np.arange(64), np.arange(64) + 64] = 1.0
    rotT[np.arange(64) + 64, np.arange(64)] = -1.0

    qp = _ab_perm(HPC)
    kp = _ab_perm(KHPC)
    wq = np.asarray(wq, dtype=np.float32)
    wk = np.asarray(wk, dtype=np.float32)
    wv = np.asarray(wv, dtype=np.float32)
    wo = np.asarray(wo, dtype=np.float32)

    w_by_hs = []
    for hs in range(TP):
        wq_c = np.ascontiguousarray(
            wq[:, hs * HPC * HD:(hs + 1) * HPC * HD][:, qp]).astype(MM_NP)
        wk_c = np.ascontiguousarray(
            wk[:, hs * KHPC * HD:(hs + 1) * KHPC * HD][:, kp]).astype(MM_NP)
        wv_c = np.ascontiguousarray(
            wv[:, hs * KHPC * HD:(hs + 1) * KHPC * HD]).astype(MM_NP)
        wo_c = np.ascontiguousarray(
            wo[hs * HPC * HD:(hs + 1) * HPC * HD, :]).astype(MM_NP)
        w_by_hs.append((wq_c, wk_c, wv_c, wo_c))

    in_maps = []
    for c in range(N_CORES):
        b, hs = divmod(c, TP)
        wq_c, wk_c, wv_c, wo_c = w_by_hs[hs]
        xT_c = np.ascontiguousarray(x[b].T).astype(MM_NP)
        in_maps.append({
            "xT": xT_c, "wq": wq_c, "wk": wk_c, "wv": wv_c, "wo": wo_c,
            "maskT": maskT, "cosT": cosT, "sinT": sinT, "rotT": rotT,
        })
    return in_maps


def kernel(x, freqs_cis, mask, wq, wk, wv, wo):
    nc = _get_compiled()
    in_maps = _host_prep(x, freqs_cis, mask, wq, wk, wv, wo)
    res = run_bass_kernel_spmd(nc, in_maps, list(range(N_CORES)))
    out = np.empty((B, S, D), dtype=np.float32)
    for b in range(B):
        out[b] = res.results[2 * b]["out"]
        out[b] += res.results[2 * b + 1]["out"]
    return out


# revision 13
# speedup vs baseline: 1.0506x; 1.0506x over previous
"""GQA causal attention block (B=4, S=1024, D=4096, H=32, KH=8, HD=128) on 8
Trainium2 NeuronCores.

Sharding: tensor-parallel over heads (2-way) x data-parallel over batch
(4-way). Core c handles batch c//2 with q-heads [16*(c%2), 16*(c%2)+16) and
kv-heads [4*(c%2), 4*(c%2)+4) over the full 1024-token sequence. Each core
produces a partial output (its heads' contribution through its wo row-slice);
the host sums the two partials per batch. No device collectives, no K/V
duplication, no fully-masked attention tiles: queries in token-group g only
attend key tiles 0..4g+3 (causal skipping, uniform across cores).

Everything stays in SBUF: x arrives pre-transposed from the host ([D, S]
fp16), Q/K/V and attention outputs are SBUF-resident (no DRAM spills, no
on-device transposes). V is projected directly into [token, head-dim] layout
by making x the stationary matmul operand. cos/sin RoPE tables come from the
host (fp16).

Pipelining: attention runs scores 2 taus ahead of the av/sum matmuls so the
in-order PE queue never waits on exp; Q-projection (g1) and wo (g0) work is
injected at attention head boundaries as PE filler. Input DMA is split across
the sync-engine queue (weights) and scalar-engine queue (x, tables, output).

RoPE trick (as baseline): wq/wk columns host-permuted per head to [even|odd]
halves; rot(q) = q*cos + (S^T q)*sin with a constant 128x128 swap matrix via
one matmul per head-chunk. Matmuls run in fp16; softmax statistics in fp32.
Scores are biased by -8 before exp (folded into mask / bias vector); the
softmax division cancels it.
"""

import numpy as np

import concourse.bass as bass
import concourse.tile as tile
from concourse import bacc, mybir
from concourse.bass_utils import run_bass_kernel_spmd

B, S, D = 4, 1024, 4096
H, KH, HD = 32, 8, 128
N_CORES = 8
TP = 2                          # head-parallel ways
HPC = H // TP                   # 16 q heads per core
KHPC = KH // TP                 # 4 kv heads per core
TG = 512                        # token-group size
G = S // TG                     # 2 token groups
SCALE = 1.0 / float(np.sqrt(HD))
EXP_BIAS = -8.0
NEG = -1e9

MM_DT = mybir.dt.float16
MM_NP = np.float16
F32 = mybir.dt.float32
BF16 = mybir.dt.bfloat16

DT = D // 128                   # 32 d-tiles
DB = 8                          # weight DMA chunks per column group
WCH = DT // DB                  # 4 d-tiles per weight chunk

_compiled = None


def _build():
    nc = bacc.Bacc("TRN2", target_bir_lowering=False, debug=False,
                   num_devices=N_CORES)

    xT = nc.dram_tensor("xT", [D, S], MM_DT, kind="ExternalInput").ap()
    wq = nc.dram_tensor("wq", [D, HPC * HD], MM_DT, kind="ExternalInput").ap()
    wk = nc.dram_tensor("wk", [D, KHPC * HD], MM_DT, kind="ExternalInput").ap()
    wv = nc.dram_tensor("wv", [D, KHPC * HD], MM_DT, kind="ExternalInput").ap()
    wo = nc.dram_tensor("wo", [HPC * HD, D], MM_DT, kind="ExternalInput").ap()
    maskT_d = nc.dram_tensor("maskT", [TG, TG], BF16, kind="ExternalInput").ap()
    cosT_d = nc.dram_tensor("cosT", [128, S], MM_DT, kind="ExternalInput").ap()
    sinT_d = nc.dram_tensor("sinT", [128, S], MM_DT, kind="ExternalInput").ap()
    rotT_d = nc.dram_tensor("rotT", [128, 128], MM_DT, kind="ExternalInput").ap()
    out = nc.dram_tensor("out", [S, D], F32, kind="ExternalOutput").ap()

    from contextlib import ExitStack

    es = ExitStack()
    with tile.TileContext(nc) as tc, es:
        const = es.enter_context(tc.tile_pool(name="const", bufs=1))
        trig = es.enter_context(tc.tile_pool(name="trig", bufs=1))
        maskp = es.enter_context(tc.tile_pool(name="maskp", bufs=1))
        xp = es.enter_context(tc.tile_pool(name="xp", bufs=1))
        qp = es.enter_context(tc.tile_pool(name="qp", bufs=2))
        kp = es.enter_context(tc.tile_pool(name="kp", bufs=1))
        vp = es.enter_context(tc.tile_pool(name="vp", bufs=1))
        atp = es.enter_context(tc.tile_pool(name="atp", bufs=2))
        wbuf = es.enter_context(tc.tile_pool(name="wbuf", bufs=5))
        pw = es.enter_context(tc.tile_pool(name="pw", bufs=3))
        sw = es.enter_context(tc.tile_pool(name="sw", bufs=2))
        pr = es.enter_context(tc.tile_pool(name="pr", bufs=3))
        rb = es.enter_context(tc.tile_pool(name="rb", bufs=1))
        ow = es.enter_context(tc.tile_pool(name="ow", bufs=2))
        ps_acc = es.enter_context(tc.tile_pool(name="ps_acc", bufs=5, space="PSUM"))
        ps_sc = es.enter_context(tc.tile_pool(name="ps_sc", bufs=3, space="PSUM"))

        # ---- x (pre-transposed on host): [128, 32, 1024], scalar queue ----
        x_t = xp.tile([128, DT, S], MM_DT, tag="x")
        for xc in range(8):
            nc.scalar.dma_start(
                out=x_t[:, xc * 4:(xc + 1) * 4, :],
                in_=xT[xc * 512:(xc + 1) * 512, :].rearrange(
                    "(dt p) t -> p dt t", p=128))

        # ---- constants / tables (scalar queue) ----
        rotT = const.tile([128, 128], MM_DT, tag="rot")
        nc.scalar.dma_start(out=rotT, in_=rotT_d)
        cosT = trig.tile([128, S], MM_DT, tag="cos")
        nc.scalar.dma_start(out=cosT, in_=cosT_d)
        sinT = trig.tile([128, S], MM_DT, tag="sin")
        nc.scalar.dma_start(out=sinT, in_=sinT_d)
        mask_t = maskp.tile([128, TG // 128, TG], BF16, tag="mask")
        nc.scalar.dma_start(
            out=mask_t, in_=maskT_d.rearrange("(t p) q -> p t q", p=128))
        ones = const.tile([128, 128], MM_DT, tag="ones")
        nc.vector.memset(ones, 1.0)
        m8_t = const.tile([128, 1], F32, tag="m8")
        nc.vector.memset(m8_t, EXP_BIAS)

        # ---- helpers ----
        def rope_evict(acc, cos_cols, sin_cols, dst):
            q_s = pw.tile([128, TG], MM_DT, tag="qs")
            nc.scalar.copy(q_s, acc)
            ps2 = ps_sc.tile([128, TG], F32, tag="sc")
            nc.tensor.matmul(ps2, rotT, q_s, start=True, stop=True)
            t1 = pw.tile([128, TG], F32, tag="t1")
            nc.vector.tensor_mul(t1, q_s, cos_cols)
            t2 = pw.tile([128, TG], F32, tag="t2")
            nc.vector.tensor_mul(t2, ps2, sin_cols)
            nc.vector.tensor_add(dst, t1, t2)

        def w_chunk(w_ap, db, c0, cols):
            w_t = wbuf.tile([128, WCH, cols], MM_DT, tag="w")
            nc.sync.dma_start(
                out=w_t,
                in_=w_ap[db * 512:(db + 1) * 512, c0:c0 + cols].rearrange(
                    "(dt p) c -> p dt c", p=128))
            return w_t

        def proj_cols(w_ap, c0, cols, tok, accs):
            """accs[jj] += w[:, c0+jj*128 : +128]^T @ x[:, tok] over all D."""
            n_jj = cols // 128
            for db in range(DB):
                w_t = w_chunk(w_ap, db, c0, cols)
                for dd in range(WCH):
                    d = db * WCH + dd
                    for jj in range(n_jj):
                        nc.tensor.matmul(
                            accs[jj], w_t[:, dd, jj * 128:(jj + 1) * 128],
                            x_t[:, d, tok], start=(d == 0), stop=(d == DT - 1))

        def make_accs(n, w=TG):
            return [ps_acc.tile([128, w], F32, tag="acc", name=f"acc{i}")
                    for i in range(n)]

        # ---- K projection + RoPE -> kT [128, 4, 1024] ----
        kT = kp.tile([128, KHPC, S], MM_DT, tag="k")
        for g in range(G):
            tok = slice(g * TG, (g + 1) * TG)
            accs = make_accs(KHPC)
            proj_cols(wk, 0, 512, tok, accs)
            for kh in range(KHPC):
                rope_evict(accs[kh], cosT[:, tok], sinT[:, tok],
                           kT[:, kh, tok])

        # ---- V projection (transposed: x stationary) -> v_n [128, 8, 512] ----
        v_n = vp.tile([128, S // 128, KHPC * HD], MM_DT, tag="v")
        for ttg in range(2):
            accs = make_accs(4, 512)
            for db in range(DB):
                w_t = w_chunk(wv, db, 0, 512)
                for dd in range(WCH):
                    d = db * WCH + dd
                    for tt in range(4):
                        tok = (ttg * 4 + tt) * 128
                        nc.tensor.matmul(
                            accs[tt], x_t[:, d, tok:tok + 128], w_t[:, dd, :],
                            start=(d == 0), stop=(d == DT - 1))
            for tt in range(4):
                nc.scalar.copy(v_n[:, ttg * 4 + tt, :], accs[tt])

        # ---- Q projection: full 4-acc groups (g0) / 2-acc halves (g1) ----
        def q_proj_cols(g, qT, c0, cols):
            tok = slice(g * TG, (g + 1) * TG)
            accs = make_accs(cols // 128)
            proj_cols(wq, c0, cols, tok, accs)
            for jj in range(cols // 128):
                rope_evict(accs[jj], cosT[:, tok], sinT[:, tok],
                           qT[:, c0 // 128 + jj, :])

        # ---- wo: one (djg, t4-pair) half; chunks persist across halves ----
        wo_chunks = {}

        def out_half(g, djg, half, attnT):
            if half == 0:
                wo_chunks[djg] = [w_chunk(wo, db, djg * 512, 512)
                                  for db in range(4)]
            accs = make_accs(2, 512)
            for db in range(4):
                w_t = wo_chunks[djg][db]
                for dd in range(WCH):
                    hd = db * WCH + dd
                    for i, t4 in enumerate((2 * half, 2 * half + 1)):
                        nc.tensor.matmul(
                            accs[i], attnT[:, hd, t4 * 128:(t4 + 1) * 128],
                            w_t[:, dd, :], start=(hd == 0),
                            stop=(hd == HPC - 1))
            for i, t4 in enumerate((2 * half, 2 * half + 1)):
                o_s = ow.tile([128, 512], F32, tag="ow")
                nc.any.tensor_copy(o_s, accs[i])
                nc.scalar.dma_start(
                    out=out[g * TG + t4 * 128:g * TG + (t4 + 1) * 128,
                            djg * 512:(djg + 1) * 512],
                    in_=o_s)

        # ---- software-pipelined attention over all heads of one group ----
        LOOKAHEAD = 2

        def attend_seq(g, qT, attnT, fillers=None):
            ntau = 4 * (g + 1)
            steps = []
            for h in range(HPC):
                st = {"h": h, "oT": None, "sum": None}
                for tau in range(ntau):
                    steps.append((st, tau))

            def issue_sc(st, tau):
                kh = st["h"] // (HPC // KHPC)
                sc_ps = ps_sc.tile([128, TG], F32, tag="sc")
                nc.tensor.matmul(
                    sc_ps, kT[:, kh, tau * 128:(tau + 1) * 128],
                    qT[:, st["h"], :], start=True, stop=True)
                p_t = pr.tile([128, TG], MM_DT, tag="pr")
                dt_ = tau - 4 * g
                if dt_ < 0:
                    nc.scalar.activation(p_t, sc_ps,
                                         mybir.ActivationFunctionType.Exp,
                                         bias=m8_t, scale=SCALE)
                else:
                    sc_s = sw.tile([128, TG], F32, tag="sw")
                    nc.vector.scalar_tensor_tensor(
                        out=sc_s, in0=sc_ps, scalar=SCALE,
                        in1=mask_t[:, dt_, :],
                        op0=mybir.AluOpType.mult, op1=mybir.AluOpType.add)
                    nc.scalar.activation(p_t, sc_s,
                                         mybir.ActivationFunctionType.Exp)
                return p_t

            def issue_av(st, tau, p_t):
                kh = st["h"] // (HPC // KHPC)
                if tau == 0:
                    st["oT"] = ps_acc.tile([128, TG], F32, tag="acc",
                                           name="oT")
                    st["sum"] = ps_acc.tile([128, TG], F32, tag="acc",
                                            name="sum")
                last = tau == ntau - 1
                nc.tensor.matmul(st["oT"],
                                 v_n[:, tau, kh * 128:(kh + 1) * 128],
                                 p_t, start=(tau == 0), stop=last)
                nc.tensor.matmul(st["sum"], ones, p_t, start=(tau == 0),
                                 stop=last)
                if last:
                    rB2 = rb.tile([128, TG], F32, tag="rb2")
                    nc.vector.reciprocal_approx_fast(rB2, st["sum"])
                    nc.vector.tensor_mul(attnT[:, st["h"], :], st["oT"], rB2)

            pend = []
            for st, tau in steps:
                p_t = issue_sc(st, tau)
                pend.append((st, tau, p_t))
                if len(pend) > LOOKAHEAD:
                    issue_av(*pend.pop(0))
                if fillers and tau == ntau - 1 and st["h"] in fillers:
                    fillers[st["h"]]()
            for item in pend:
                issue_av(*item)

        # ---- phase schedule ----
        qT0 = qp.tile([128, HPC, TG], MM_DT, tag="q", name="qT0")
        qT1 = qp.tile([128, HPC, TG], MM_DT, tag="q", name="qT1")
        attnT0 = atp.tile([128, HPC, TG], MM_DT, tag="at", name="at0")
        attnT1 = atp.tile([128, HPC, TG], MM_DT, tag="at", name="at1")

        # Q for g0 (full 512-col groups)
        for jg in range(4):
            q_proj_cols(0, qT0, jg * 512, 512)
        # attention g0 with Q-g1 256-col half-groups as PE fillers
        attend_seq(0, qT0, attnT0, fillers={
            2 * u + 1: (lambda u=u: q_proj_cols(1, qT1, u * 256, 256))
            for u in range(8)})
        # attention g1 with wo-g0 (djg, t4-pair) halves as PE fillers
        attend_seq(1, qT1, attnT1, fillers={
            h: (lambda h=h: out_half(0, h // 2, h % 2, attnT0))
            for h in range(HPC)})
        # wo for g1 (full 4-acc groups)
        def out_group(g, djg, attnT):
            accs = make_accs(4, 512)
            for db in range(4):
                w_t = w_chunk(wo, db, djg * 512, 512)
                for dd in range(WCH):
                    hd = db * WCH + dd
                    for t4 in range(4):
                        nc.tensor.matmul(
                            accs[t4], attnT[:, hd, t4 * 128:(t4 + 1) * 128],
                            w_t[:, dd, :], start=(hd == 0),
                            stop=(hd == HPC - 1))
            for t4 in range(4):
                o_s = ow.tile([128, 512], F32, tag="ow")
                nc.any.tensor_copy(o_s, accs[t4])
                nc.scalar.dma_start(
                    out=out[g * TG + t4 * 128:g * TG + (t4 + 1) * 128,
                            djg * 512:(djg + 1) * 512],
                    in_=o_s)

        for djg in range(8):
            out_group(1, djg, attnT1)

    nc.compile()
    return nc


def _get_compiled():
    global _compiled
    if _compiled is None:
        _compiled = _build()
    return _compiled


def _ab_perm(n_heads):
    """Per-head column permutation to [even dims | odd dims]."""
    p = []
    for h in range(n_heads):
        base = h * HD
        p.extend(range(base, base + HD, 2))
        p.extend(range(base + 1, base + HD, 2))
    return np.asarray(p)


def _host_prep(x, freqs_cis, mask, wq, wk, wv, wo):
    """Shard per core: core c -> batch c//2, head-slice c%2."""
    import ml_dtypes

    x = np.asarray(x, dtype=np.float32)

    # RoPE tables [128, S] fp16: rows 0:64 = trig(pos*inv_freq_i), 64:128 dup
    fr = np.asarray(freqs_cis, dtype=np.float64)      # [S, 64]
    cosT = np.empty((128, S), dtype=MM_NP)
    sinT = np.empty((128, S), dtype=MM_NP)
    cosT[0:64] = np.cos(fr).T
    cosT[64:128] = cosT[0:64]
    sinT[0:64] = np.sin(fr).T
    sinT[64:128] = sinT[0:64]

    # causal 512-block mask [keys, queries] with -8 exp-bias folded in
    kk, qq = np.meshgrid(np.arange(TG), np.arange(TG), indexing="ij")
    maskT = np.where(kk <= qq, np.float32(EXP_BIAS), np.float32(NEG))
    maskT = maskT.astype(ml_dtypes.bfloat16)

    # S^T for rot(q) = q*cos + (S^T q)*sin in [a|b] layout
    rotT = np.zeros((128, 128), dtype=MM_NP)
    rotT[np.arange(64), np.arange(64) + 64] = 1.0
    rotT[np.arange(64) + 64, np.arange(64)] = -1.0

    qperm = _ab_perm(HPC)
    kperm = _ab_perm(KHPC)
    wq = np.asarray(wq, dtype=np.float32)
    wk = np.asarray(wk, dtype=np.float32)
    wv = np.asarray(wv, dtype=np.float32)
    wo = np.asarray(wo, dtype=np.float32)

    w_by_hs = []
    for hs in range(TP):
        wq_c = np.ascontiguousarray(
            wq[:, hs * HPC * HD:(hs + 1) * HPC * HD][:, qperm]).astype(MM_NP)
        wk_c = np.ascontiguousarray(
            wk[:, hs * KHPC * HD:(hs + 1) * KHPC * HD][:, kperm]).astype(MM_NP)
        wv_c = np.ascontiguousarray(
            wv[:, hs * KHPC * HD:(hs + 1) * KHPC * HD]).astype(MM_NP)
        wo_c = np.ascontiguousarray(
            wo[hs * HPC * HD:(hs + 1) * HPC * HD, :]).astype(MM_NP)
        w_by_hs.append((wq_c, wk_c, wv_c, wo_c))

    in_maps = []
    for c in range(N_CORES):
        b, hs = divmod(c, TP)
        wq_c, wk_c, wv_c, wo_c = w_by_hs[hs]
        xT_c = np.ascontiguousarray(x[b].T).astype(MM_NP)
        in_maps.append({
            "xT": xT_c, "wq": wq_c, "wk": wk_c, "wv": wv_c, "wo": wo_c,
            "maskT": maskT, "cosT": cosT, "sinT": sinT, "rotT": rotT,
        })
    return in_maps


def kernel(x, freqs_cis, mask, wq, wk, wv, wo):
    nc = _get_compiled()
    in_maps = _host_prep(x, freqs_cis, mask, wq, wk, wv, wo)
    res = run_bass_kernel_spmd(nc, in_maps, list(range(N_CORES)))
    out = np.empty((B, S, D), dtype=np.float32)
    for b in range(B):
        out[b] = res.results[2 * b]["out"]
        out[b] += res.results[2 * b + 1]["out"]
    return out
